# revision 48
# baseline (speedup 1.0000x reference)
"""Trainium2 Bass kernel for nn_AttentionBlock (B=16, C=512, H=W=32).

Math (verified exact vs reference, rel err 3e-9 in fp64/fp32):
  - GroupNorm(32, eps=1e-5), no affine -> hn [C, P], P = H*W flat (h*32+w).
  - The torch einsum `bHWHW,bcWH->bcWH` takes the softmax DIAGONAL, so all
    that survives of the attention is a per-position scale
        d[p=32h+w] = diagT[h, w],
        diagT[i,j] = 1024*exp(sc*S[33i,33j]) / sum_{h1,h2} exp(sc*S[32h1+i, 32h2+j])
    with S = hn^T (Wq Wk^T) hn, sc = C^-0.5 (the 1024 = position-count fold).
  - out = x + (1/65536) * (64*WvWn)^T (hn * d_bcast)   [weights pre-scaled x64
    for fp8 dynamic range; 65536 = 64*1024 unfolds both scales]

Precision: all big matmuls run fp8e4 (DoubleRow, K=256/instr). The attention
correction is ~2e-4 of ||x||, so a few % of fp8 noise on it is ~1e-5 overall.

Sharding: data-parallel over batch, 2 per core, no collectives.
"""

import math
import os
import sys

import numpy as np

for _p in ("/opt/trn_rl_repo", "/opt/pypackages"):
    if os.path.isdir(_p) and _p not in sys.path:
        sys.path.append(_p)

import concourse.bass as bass
import concourse.mybir as mybir
import concourse.tile as tile
from concourse.bass_utils import run_bass_kernel_spmd

B, C, H, W = 16, 512, 32, 32
NPOS = H * W            # 1024
NCORES = 8
BPC = B // NCORES       # batches per core
KT = 4                  # 512 channels = 4 k-tiles of 128
EPS = 1e-5
SC = float(C) ** -0.5
WSCALE = 64.0           # host pre-scale on G / WVN for fp8 range
EXP_SCALE = SC / WSCALE
LN1024 = math.log(1024.0)
OUT_SCALE = 1.0 / (WSCALE * 1024.0)
XSCALE = 65536.0        # host pre-scale on x (= 1/OUT_SCALE, exact pow2)
EPS_DEV = EPS * XSCALE * XSCALE
F32 = mybir.dt.float32
F32R = mybir.dt.float32r
FP8 = mybir.dt.float8e4
AF = mybir.ActivationFunctionType
ALU = mybir.AluOpType
AX = mybir.AxisListType
DR = mybir.MatmulPerfMode.DoubleRow

# aux constant-tensor (fp32) column layout
A_F16 = 0             # [128, 8]    F16[p, g] = (p // 16 == g) / 16
A_E16 = 8             # [8, 128]    E16[g, q] = (q // 16 == g)
A_I128 = 136          # [128, 128]  identity (residual matmul)
A_ONES32 = 264        # [32, 128]   ones (diag broadcast matmul, K=32)
NAUXF = 392
# fp8 merged-const column layout (bytes)
Q_G = 0               # [128, 4*512] g rearranged (k p) n -> p (k n)
Q_WVN = 2048
Q_FIND = 4096         # [128, 2*32] f_ind pair
Q_R32H = 4160         # [32, 1024]  R32H[k, n] = (n // 32 == k)  (0/1, exact fp8)
NQ = 5184


def _r(ap):
    return ap.bitcast(F32R)


def _split_sync_waits(nc, maxw=1):
    """walrus embeds at most one sync-wait per instruction; move extra waits
    onto preceding same-queue NoOps (FIFO queues keep semantics)."""
    n = 0
    for fn in nc.m.functions:
        for blk in fn.blocks:
            out = []
            for inst in blk.instructions:
                si = inst.sync_info
                waits = list(si.on_wait) if (si is not None and si.on_wait) else []
                if len(waits) > maxw:
                    keep = waits[-maxw:]
                    extra = waits[:-maxw]
                    for i in range(0, len(extra), maxw):
                        nop = mybir.InstNoOp(name=f"wsplit-{n}")
                        n += 1
                        nop.engine = inst.engine
                        nop.sync_info = mybir.SyncInfo(
                            on_wait=extra[i:i + maxw], on_update=[]
                        )
                        out.append(nop)
                    si.on_wait = keep
                out.append(inst)
            blk.instructions = out
    return n


def _build_nc():
    nc = bass.Bass()
    x_ext = nc.declare_dram_parameter("x", [BPC, C, NPOS], F32, isOutput=False)
    aux_ext = nc.declare_dram_parameter("aux", [128, NAUXF], F32, isOutput=False)
    fq_ext = nc.declare_dram_parameter("fq", [128, NQ], FP8, isOutput=False)
    out_ext = nc.declare_dram_parameter("out", [BPC, C, NPOS], F32, isOutput=True)

    with tile.TileContext(nc) as tc:
        from contextlib import ExitStack

        with ExitStack() as ctx:
            wpool = ctx.enter_context(tc.tile_pool(name="wpool", bufs=1))
            xpool = ctx.enter_context(tc.tile_pool(name="xpool", bufs=2))
            hnpool = ctx.enter_context(tc.tile_pool(name="hnpool", bufs=2))
            hhpool = ctx.enter_context(tc.tile_pool(name="hhpool", bufs=2))
            hspool = ctx.enter_context(tc.tile_pool(name="hspool", bufs=2))
            opool = ctx.enter_context(tc.tile_pool(name="opool", bufs=2))
            epool = ctx.enter_context(tc.tile_pool(name="epool", bufs=4))
            spool = ctx.enter_context(tc.tile_pool(name="spool", bufs=2))
            ps_big = ctx.enter_context(tc.tile_pool(name="ps_big", bufs=2, space="PSUM"))
            ps_r = ctx.enter_context(tc.tile_pool(name="ps_r", bufs=1, space="PSUM"))
            ps_sm = ctx.enter_context(tc.tile_pool(name="ps_sm", bufs=2, space="PSUM"))

            fq_sb = wpool.tile([128, NQ], FP8, tag="fq_sb", name="fq_sb")
            aux_sb = wpool.tile([128, NAUXF], F32R, tag="aux_sb", name="aux_sb")
            warm_sb = wpool.tile([128, 512], F32, tag="warm_sb", name="warm_sb")
            eps_sb = wpool.tile([128, 1], F32, tag="eps_sb", name="eps_sb")
            ln1024_sb = wpool.tile([128, 1], F32, tag="ln1024_sb", name="ln1024_sb")

            g_sb = fq_sb[:, Q_G:Q_G + 2048].rearrange("p (k n) -> p k n", k=KT)
            wvn_sb = fq_sb[:, Q_WVN:Q_WVN + 2048].rearrange("p (k n) -> p k n", k=KT)
            auxq_sb = fq_sb[:, Q_FIND:Q_FIND + 64].rearrange("p (a b) -> p a b", a=2)
            f16 = aux_sb[:, A_F16:A_F16 + 8]
            e16 = aux_sb[0:8, A_E16:A_E16 + 128]
            i128 = aux_sb[:, A_I128:A_I128 + 128]
            ones32 = aux_sb[0:32, A_ONES32:A_ONES32 + 128]
            r32h = fq_sb[0:32, Q_R32H:Q_R32H + NPOS]

            st = [dict() for _ in range(BPC)]

            def warmup(n):
                nc.vector.memset(warm_sb, 0.0)
                nc.vector.memset(eps_sb, EPS_DEV)
                nc.vector.memset(ln1024_sb, LN1024)
                # dense N=512 stream: ~80% PE duty, flips the HAM clock gate
                # to 2.4GHz ~4us in (N=128 with per-MM LDWEIGHTS never does)
                wps = ps_sm.tile([128, 512], F32, tag="sm", name="sm")
                for _ in range(n):
                    nc.tensor.matmul(wps, _r(warm_sb[:, 0:128]), _r(warm_sb),
                                     start=True, stop=True)

            def filler_f32(n, rhs_ap):
                """Keep-warm matmuls gated on real data (bridges DMA waits so
                the HAM clock gate never re-throttles the PE)."""
                wf = ps_big.tile([128, 512], F32, tag="big", name="big")
                for _ in range(n):
                    nc.tensor.matmul(wf, _r(warm_sb[:, 0:128]), rhs_ap,
                                     start=True, stop=True)

            def filler_ep(n, ep):
                wf = ps_big.tile([128, 512], F32, tag="big", name="big")
                for _ in range(n):
                    nc.tensor.matmul(wf[0:32, 0:256], auxq_sb, ep[:, :, 0:256],
                                     start=True, stop=True, perf_mode=DR)

            def load_input_dmas():
                """Few big DMAs: each HWDGE transfer costs ~2.4us of serial
                ring time, so transfer count dominates the ramp."""
                for b in range(BPC):
                    st[b]["x"] = [
                        xpool.tile([128, 2, NPOS], F32R, tag=f"x_sb{h}",
                                   name=f"x_sb{h}")
                        for h in range(2)
                    ]
                # ring ACT: fp8 consts + aux (small, done early)
                nc.scalar.dma_start(out=fq_sb, in_=fq_ext[:, :])
                nc.scalar.dma_start(out=aux_sb, in_=aux_ext[:, :].bitcast(F32R))
                # ring SP: all of x, batch 0 first (it gates the whole ramp)
                xv0 = x_ext[0].bitcast(F32R).rearrange("(h p) n -> p h n", p=128)
                nc.sync.dma_start(out=st[0]["x"][0], in_=xv0[:, 0:2])
                nc.sync.dma_start(out=st[0]["x"][1], in_=xv0[:, 2:4])
                xv1 = x_ext[1].bitcast(F32R).rearrange("(h p) n -> p h n", p=128)
                nc.sync.dma_start(out=st[1]["x"][0], in_=xv1[:, 0:2])
                nc.sync.dma_start(out=st[1]["x"][1], in_=xv1[:, 2:4])

            def xkt(b, kt):
                return st[b]["x"][kt // 2][:, kt % 2]

            def stats_pair(b, h):
                """GroupNorm stats + fp8 hn cast for one x half (kts 2h,2h+1).
                Gates only on that half's DMA, so kt01 normalizes while kt23
                is still in flight."""
                s = st[b]
                if h == 0:
                    s["hn"] = hnpool.tile([128, KT, NPOS], FP8, tag="hn_sb",
                                          name="hn_sb")
                hn_sb = s["hn"]
                stats = spool.tile([128, 2, 2, 6], F32, tag=f"stats{h}",
                                   name=f"stats{h}")
                for j in range(2):
                    for sub in range(2):
                        nc.vector.bn_stats(
                            out=stats[:, j, sub, :],
                            in_=xkt(b, 2 * h + j)[:, sub * 512:(sub + 1) * 512]
                            .bitcast(F32),
                        )
                mv = spool.tile([128, 2, 2], F32, tag=f"mv{h}", name=f"mv{h}")
                for j in range(2):
                    nc.vector.bn_aggr(out=mv[:, j, :], in_=stats[:, j, :, :])
                rhs4 = spool.tile([128, 4], F32R, tag=f"rhs4_{h}",
                                  name=f"rhs4_{h}")
                nc.vector.tensor_copy(out=rhs4[:, 0:2], in_=mv[:, :, 0])
                nc.vector.tensor_tensor(
                    out=rhs4[:, 2:4], in0=mv[:, :, 0], in1=mv[:, :, 0], op=ALU.mult
                )
                nc.vector.tensor_tensor(
                    out=rhs4[:, 2:4], in0=rhs4[:, 2:4].bitcast(F32), in1=mv[:, :, 1],
                    op=ALU.add,
                )
                gst_ps = ps_sm.tile([8, 4], F32, tag="sm", name="sm")
                nc.tensor.matmul(gst_ps, _r(f16), _r(rhs4), start=True, stop=True)
                # mu_inv: cols 0:2 = -mu_g, cols 2:4 = invsigma_g (per kt)
                mu_inv = spool.tile([8, 4], F32R, tag=f"mu_inv{h}",
                                    name=f"mu_inv{h}")
                nc.scalar.mul(out=mu_inv[:, 0:2], in_=gst_ps[:, 0:2], mul=-1.0)
                m2 = spool.tile([8, 2], F32, tag=f"m2_{h}", name=f"m2_{h}")
                nc.scalar.square(out=m2, in_=gst_ps[:, 0:2])
                var2 = spool.tile([8, 2], F32, tag=f"var2_{h}", name=f"var2_{h}")
                nc.vector.tensor_tensor(
                    out=var2, in0=gst_ps[:, 2:4], in1=m2, op=ALU.subtract
                )
                lnv = spool.tile([8, 2], F32, tag=f"lnv{h}", name=f"lnv{h}")
                nc.scalar.activation(out=lnv, in_=var2, func=AF.Ln,
                                     bias=eps_sb[0:8, :])
                nc.scalar.activation(out=mu_inv[:, 2:4], in_=lnv, func=AF.Exp,
                                     scale=-0.5)
                perch_ps = ps_sm.tile([128, 4], F32, tag="sm", name="sm")
                nc.tensor.matmul(perch_ps, _r(e16), _r(mu_inv), start=True,
                                 stop=True)
                perch = spool.tile([128, 4], F32, tag=f"perch{h}",
                                   name=f"perch{h}")
                nc.vector.tensor_copy(out=perch, in_=perch_ps)
                negms = spool.tile([128, 2], F32, tag=f"negms{h}",
                                   name=f"negms{h}")
                nc.vector.tensor_tensor(
                    out=negms, in0=perch[:, 0:2], in1=perch[:, 2:4], op=ALU.mult
                )
                for j in range(2):
                    kt = 2 * h + j
                    if j == 1 or b == 1:
                        nc.vector.tensor_scalar(
                            out=hn_sb[:, kt],
                            in0=xkt(b, kt).bitcast(F32),
                            scalar1=perch[:, 2 + j:3 + j],
                            scalar2=negms[:, j:j + 1],
                            op0=ALU.mult,
                            op1=ALU.add,
                        )
                    else:
                        nc.scalar.activation(
                            out=hn_sb[:, kt],
                            in_=xkt(b, kt).bitcast(F32),
                            func=AF.Identity,
                            scale=perch[:, 2 + j:3 + j],
                            bias=negms[:, j:j + 1],
                        )

            def hhat_mt(b, mt):
                """hh[mt] = (64*G[:,mt])^T hn, fp8 DoubleRow, cast to fp8."""
                s = st[b]
                hn_sb = s["hn"]
                if mt == 0:
                    s["hh"] = hhpool.tile([128, KT, NPOS], FP8, tag="hh_sb",
                                          name="hh_sb")
                hh_sb = s["hh"]
                ps = ps_big.tile([128, NPOS], F32, tag="big", name="big")
                # g-major: the g=0 matmuls only need hn kt0/1, so they can
                # start while the last x chunks are still landing
                for g in range(2):
                    for nh in range(2):
                        sl = slice(nh * 512, (nh + 1) * 512)
                        nc.tensor.matmul(
                            ps[:, sl],
                            g_sb[:, 2 * g:2 * g + 2, mt * 128:(mt + 1) * 128],
                            hn_sb[:, 2 * g:2 * g + 2, sl],
                            start=(g == 0),
                            stop=(g == 1),
                            perf_mode=DR,
                        )
                if mt % 2 == 0:
                    nc.scalar.copy(out=hh_sb[:, mt, :], in_=ps)
                else:
                    nc.vector.tensor_copy(out=hh_sb[:, mt, :], in_=ps)

            def hhat(b):
                for mt in range(KT):
                    hhat_mt(b, mt)

            def numer(b):
                """numT = 1024*exp(sc*S[33i,33j]) via strided fp8 matmul."""
                s = st[b]
                hn_sb, hh_sb = s["hn"], s["hh"]
                nps = ps_sm.tile([32, 32], F32, tag="sm", name="sm")
                for kt in range(KT):
                    nc.tensor.matmul(
                        nps,
                        hh_sb[:, kt, 0:NPOS:33],
                        hn_sb[:, kt, 0:NPOS:33],
                        start=(kt == 0),
                        stop=(kt == KT - 1),
                    )
                s["numT"] = numT = spool.tile([32, 32], F32, tag="numT", name="numT")
                nc.scalar.activation(out=numT, in_=nps, func=AF.Exp,
                                     scale=EXP_SCALE, bias=ln1024_sb[0:32, :])

            def s_phase(b, post_nt):
                """S tiles -> exp(fp8) -> psR row-reduction (2-tile lag so the
                psR matmul never stalls the PE on the exp)."""
                s = st[b]
                hn_sb, hh_sb = s["hn"], s["hh"]
                s["psR"] = psR = ps_r.tile([32, NPOS], F32, tag="psR", name="psR")
                pairs = []
                e_pair = None

                def psr_mm(pi):
                    ep = pairs[pi]
                    for mh in range(2):
                        sl = slice(mh * 512, (mh + 1) * 512)
                        nc.tensor.matmul(
                            psR[:, sl],
                            auxq_sb,
                            ep[:, :, sl],
                            start=(pi == 0),
                            stop=(pi == 3),
                            perf_mode=DR,
                            skip_group_check=True,
                        )

                for nt in range(8):
                    ps = ps_big.tile([128, NPOS], F32, tag="big", name="big")
                    # g-major: one LDWEIGHTS serves both mh halves (PE duty up)
                    for g in range(2):
                        for mh in range(2):
                            sl = slice(mh * 512, (mh + 1) * 512)
                            nc.tensor.matmul(
                                ps[:, sl],
                                hh_sb[:, 2 * g:2 * g + 2, nt * 128:(nt + 1) * 128],
                                hn_sb[:, 2 * g:2 * g + 2, sl],
                                start=(g == 0),
                                stop=(g == 1),
                                perf_mode=DR,
                            )
                    if nt % 2 == 0:
                        e_pair = epool.tile([128, 2, NPOS], FP8, tag="e_pair",
                                            name="e_pair")
                        pairs.append(e_pair)
                    nc.scalar.activation(out=e_pair[:, nt % 2, :], in_=ps,
                                         func=AF.Exp, scale=EXP_SCALE)
                    for fn in post_nt.get(nt, []):
                        fn()
                for pi in range(4):
                    psr_mm(pi)
                s["last_ep"] = pairs[3]

            def diag_chain(b):
                """psR(PSUM) -> denT -> diagT (exact orientation, no transpose)."""
                s = st[b]
                psR, numT = s["psR"], s["numT"]
                denT = spool.tile([32, 32], F32, tag="denT", name="denT")
                nc.vector.tensor_reduce(
                    out=denT,
                    in_=psR.rearrange("p (a b) -> p b a", a=32),
                    axis=AX.X,
                    op=ALU.add,
                )
                rdenT = spool.tile([32, 32], F32, tag="rdenT", name="rdenT")
                nc.vector.reciprocal(out=rdenT, in_=denT)
                diagT = spool.tile([32, 32], F32, tag="diagT", name="diagT")
                nc.vector.tensor_tensor(out=diagT, in0=numT, in1=rdenT, op=ALU.mult)
                s["diagT"] = diagT

            def d_bcast_half(b, nh):
                """D[c, n] = diagT[n//32, n%32] broadcast: mask-multiply on the
                DVE (stride-0 broadcast read) + K=32 ones matmul. Avoids the
                SBUF->SBUF flatten DMA (~2.4us of ring serial time)."""
                s = st[b]
                diagT = s["diagT"]
                sl = slice(nh * 512, (nh + 1) * 512)
                masked = spool.tile([32, 512], F32R, tag=f"msk{nh}",
                                    name=f"msk{nh}")
                nc.vector.tensor_tensor(
                    out=masked.rearrange("p (a b) -> p a b", a=16),
                    in0=r32h[:, sl].rearrange("p (a b) -> p a b", a=16),
                    in1=diagT.unsqueeze(1).broadcast_to([32, 16, 32]),
                    op=ALU.mult,
                )
                ps_d = ps_sm.tile([128, 512], F32, tag="sm", name="sm")
                nc.tensor.matmul(ps_d, _r(ones32), masked, start=True, stop=True)
                s.setdefault("ps_d", [None, None])[nh] = ps_d

            def hs_half(b, nh):
                """hs[:, :, half] = hn * D (fp8), 4 DVE ops."""
                s = st[b]
                hn_sb = s["hn"]
                ps_d = s["ps_d"][nh]
                if nh == 0:
                    s["hs"] = hspool.tile([128, KT, NPOS], FP8, tag="hs_sb",
                                          name="hs_sb")
                hs_sb = s["hs"]
                sl = slice(nh * 512, (nh + 1) * 512)
                for kt in range(KT):
                    nc.vector.tensor_tensor(
                        out=hs_sb[:, kt, sl], in0=hn_sb[:, kt, sl], in1=ps_d,
                        op=ALU.mult,
                    )

            def out_mt(b, mt, residual_pe):
                """psum[mt] = (64*WVN)^T hs (+ 65536*x via identity matmul on
                the tail path); lands in the o2 pair tile via ACT/DVE."""
                s = st[b]
                hs_sb = s["hs"]
                ps = ps_big.tile([128, NPOS], F32, tag="big", name="big")
                for g in range(2):
                    for nh in range(2):
                        sl = slice(nh * 512, (nh + 1) * 512)
                        nc.tensor.matmul(
                            ps[:, sl],
                            wvn_sb[:, 2 * g:2 * g + 2, mt * 128:(mt + 1) * 128],
                            hs_sb[:, 2 * g:2 * g + 2, sl],
                            start=(g == 0),
                            stop=(not residual_pe and g == 1),
                            perf_mode=DR,
                        )
                if residual_pe:
                    for nh in range(2):
                        sl = slice(nh * 512, (nh + 1) * 512)
                        nc.tensor.matmul(
                            ps[:, sl], _r(i128), xkt(b, mt)[:, sl],
                            start=False, stop=True,
                        )
                if mt % 2 == 0:
                    s["o2"] = opool.tile([128, 2, NPOS], F32, tag="o_sb",
                                         name="o_sb")
                o2 = s["o2"]
                if residual_pe:
                    # halves on both engines in parallel
                    nc.scalar.copy(out=o2[:, mt % 2, 0:512], in_=ps[:, 0:512])
                    nc.vector.tensor_copy(out=o2[:, mt % 2, 512:1024],
                                          in_=ps[:, 512:1024])
                else:
                    nc.vector.tensor_tensor(out=o2[:, mt % 2, :], in0=ps,
                                            in1=xkt(b, mt).bitcast(F32),
                                            op=ALU.add)

            def out_dma(b, pair, ring):
                """One 1MB DMA per mt pair (each transfer costs ~2.4us ring)."""
                s = st[b]
                ov = out_ext[b].rearrange("(k p) n -> p k n", p=128)
                ring.dma_start(out=ov[:, 2 * pair:2 * pair + 2, :], in_=s["o2"])

            def out_dma_single(b, mt, ring):
                """0.5MB DMA for one mt (smaller final transfer on the tail)."""
                s = st[b]
                ov = out_ext[b].rearrange("(k p) n -> p k n", p=128)
                ring.dma_start(out=ov[:, mt:mt + 1, :],
                               in_=s["o2"][:, mt % 2:mt % 2 + 1, :])

            # ---- pipelined emission over the two batches ----
            warmup(int(os.environ.get("TRN_WARM_N", "24")))
            load_input_dmas()
            stats_pair(0, 0)
            filler_f32(6, xkt(0, 0)[:, 0:512])    # keep HAM warm to x0b
            stats_pair(0, 1)
            filler_f32(6, xkt(0, 2)[:, 0:512])    # keep HAM warm to hn-casts
            hhat(0)
            # batch-1 stats interleave into batch-0's S phase
            numer(0)
            # batch-1 stats + hhat + numer all absorb into batch-0's S phase
            s_phase(0, post_nt={
                1: [lambda: stats_pair(1, 0)],
                2: [lambda: stats_pair(1, 1)],
                6: [lambda: hhat_mt(1, 0), lambda: hhat_mt(1, 1)],
                7: [lambda: hhat_mt(1, 2), lambda: hhat_mt(1, 3),
                    lambda: numer(1)],
            })
            diag_chain(0)
            # batch-1 S phase interleaved with batch-0 hs/out phase
            s_phase(1, post_nt={
                0: [lambda: d_bcast_half(0, 0), lambda: hs_half(0, 0),
                    lambda: d_bcast_half(0, 1), lambda: hs_half(0, 1)],
                3: [lambda: out_mt(0, 0, False)],
                5: [lambda: out_mt(0, 1, False),
                    lambda: out_dma(0, 0, nc.sync)],
                6: [lambda: out_mt(0, 2, False)],
                7: [lambda: out_mt(0, 3, True),
                    lambda: out_dma(0, 1, nc.sync)],
            })
            filler_ep(8, st[1]["last_ep"])        # bridge diag1 chain
            diag_chain(1)
            # tail: PE-residual + ACT copy keeps the DVE off the critical path
            d_bcast_half(1, 0)
            hs_half(1, 0)
            filler_ep(8, st[1]["last_ep"])        # bridge hs latency, stay warm
            d_bcast_half(1, 1)
            hs_half(1, 1)
            out_mt(1, 0, True)
            out_mt(1, 1, True)
            out_dma(1, 0, nc.sync)
            out_mt(1, 2, True)
            out_dma_single(1, 2, nc.sync)
            out_mt(1, 3, True)
            out_dma_single(1, 3, nc.scalar)
    if os.environ.get("TRN_NO_WAITSPLIT") != "1":
        _split_sync_waits(nc, maxw=1)
    return nc


def _make_aux():
    aux = np.zeros((128, NAUXF), np.float32)
    p = np.arange(128)
    aux[p, A_F16 + (p // 16) % 8] = 1.0 / 16.0
    for g in range(8):
        for q in range(128):
            if q // 16 == g:
                aux[g, A_E16 + q] = 1.0
    aux[p, A_I128 + p] = 1.0
    aux[0:32, A_ONES32:A_ONES32 + 128] = 1.0
    return aux


def _make_fq(G, WVN, FP8NP):
    """Merged fp8 consts: g / wvn rearranged (k p) n -> p (k n), f_ind pair."""
    fq = np.zeros((128, NQ), FP8NP)
    gr = G.reshape(KT, 128, C).transpose(1, 0, 2).reshape(128, KT * C)
    wr = WVN.reshape(KT, 128, C).transpose(1, 0, 2).reshape(128, KT * C)
    fq[:, Q_G:Q_G + 2048] = gr
    fq[:, Q_WVN:Q_WVN + 2048] = wr
    p = np.arange(128)
    fq[p, Q_FIND + p % 32] = 1.0
    fq[p, Q_FIND + 32 + p % 32] = 1.0
    n = np.arange(NPOS)
    for k in range(32):
        fq[k, Q_R32H:Q_R32H + NPOS] = (n // 32 == k).astype(np.float32)
    return fq


def _reference_numpy(x, Wq, bq, Wk, bk, Wv, bv, Wn, bn):
    """Exact (slow) numpy fallback, only used if biases are nonzero."""
    Bn_, C_, H_, W_ = x.shape
    xg = x.reshape(Bn_, 32, -1).astype(np.float64)
    mu = xg.mean(-1, keepdims=True)
    var = xg.var(-1, keepdims=True)
    h = ((xg - mu) / np.sqrt(var + EPS)).reshape(Bn_, C_, H_, W_).astype(np.float32)
    bqv = bq.reshape(1, C_, 1, 1)
    bkv = bk.reshape(1, C_, 1, 1)
    bvv = bv.reshape(1, C_, 1, 1)
    bnv = bn.reshape(1, C_, 1, 1)

    def nin(t, Wm, bb):
        return np.einsum("bchw,co->bowh", t, Wm, optimize=True) + bb

    q = nin(h, Wq, bqv)
    k = nin(h, Wk, bkv)
    v = nin(h, Wv, bvv)
    out = np.empty_like(x)
    sc = C_ ** -0.5
    for bi in range(Bn_):
        Q = q[bi].transpose(2, 1, 0).reshape(-1, C_)
        K = k[bi].transpose(2, 1, 0).reshape(-1, C_)
        S = (Q @ K.T) * sc
        S5 = S.reshape(H_, W_, H_, W_).transpose(1, 3, 0, 2)
        Sm = S5.reshape(W_, W_, -1)
        Sm = Sm - Sm.max(-1, keepdims=True)
        E = np.exp(Sm)
        SMX = (E / E.sum(-1, keepdims=True)).reshape(W_, W_, H_, H_)
        ii = np.arange(H_)
        jj = np.arange(W_)
        diag = SMX[ii[:, None], jj[None, :], ii[:, None], jj[None, :]]
        h2v = v[bi] * np.swapaxes(diag, 0, 1)[None]
        out[bi] = np.einsum("cwh,co->ohw", h2v, Wn, optimize=True) + bnv[0]
    return (x + out).astype(np.float32)


_NC_CACHE = None


def kernel(**inputs):
    x = np.ascontiguousarray(np.asarray(inputs["x"], dtype=np.float32))
    Wq = np.asarray(inputs["Wq"], dtype=np.float32)
    Wk = np.asarray(inputs["Wk"], dtype=np.float32)
    Wv = np.asarray(inputs["Wv"], dtype=np.float32)
    Wn = np.asarray(inputs["Wn"], dtype=np.float32)
    bq = np.asarray(inputs["bq"], dtype=np.float32)
    bk = np.asarray(inputs["bk"], dtype=np.float32)
    bv = np.asarray(inputs["bv"], dtype=np.float32)
    bn = np.asarray(inputs["bn"], dtype=np.float32)

    if any(np.any(bb != 0) for bb in (bq, bk, bv, bn)):
        return _reference_numpy(x, Wq, bq, Wk, bk, Wv, bv, Wn, bn)

    import ml_dtypes

    FP8NP = ml_dtypes.float8_e4m3
    G = np.clip(Wq @ Wk.T * WSCALE, -240, 240).astype(FP8NP)
    WVN = np.clip(Wv @ Wn * WSCALE, -240, 240).astype(FP8NP)
    aux = _make_aux()
    fq = _make_fq(G, WVN, FP8NP)

    global _NC_CACHE
    if _NC_CACHE is None:
        _NC_CACHE = _build_nc()
    nc = _NC_CACHE

    xf = (x * XSCALE).reshape(B, C, NPOS)   # exact pow2 scale, undone on device
    in_maps = [
        {
            "x": np.ascontiguousarray(xf[c * BPC:(c + 1) * BPC]),
            "aux": aux,
            "fq": fq,
        }
        for c in range(NCORES)
    ]
    trace = bool(int(os.environ.get("TRN_KERNEL_TRACE", "0")))
    res = run_bass_kernel_spmd(nc, in_maps, core_ids=list(range(NCORES)), trace=trace)
    if trace:
        kernel.last_exec_time_ns = res.exec_time_ns
        kernel.last_results = res
    out = np.empty((B, C, NPOS), np.float32)
    for c in range(NCORES):
        # device emits 65536*(x + correction); undo the exact pow2 scale
        out[c * BPC:(c + 1) * BPC] = res.results[c]["out"]
    out *= OUT_SCALE
    return out.reshape(B, C, H, W)


# revision 49
# speedup vs baseline: 1.1617x; 1.1617x over previous
"""Trainium2 Bass kernel for nn_AttentionBlock (B=16, C=512, H=W=32).

Math (verified exact vs reference, rel err 3e-9 in fp64/fp32):
  - GroupNorm(32, eps=1e-5), no affine -> hn [C, P], P = H*W flat (h*32+w).
  - The torch einsum `bHWHW,bcWH->bcWH` takes the softmax DIAGONAL, so all
    that survives of the attention is a per-position scale
        d[p=32h+w] = diagT[h, w],
        diagT[i,j] = 1024*exp(sc*S[33i,33j]) / sum_{h1,h2} exp(sc*S[32h1+i, 32h2+j])
    with S = hn^T (Wq Wk^T) hn, sc = C^-0.5 (the 1024 = position-count fold).
  - out = x + (1/65536) * (64*WvWn)^T (hn * d_bcast)   [weights pre-scaled x64
    for fp8 dynamic range; 65536 = 64*1024 unfolds both scales]

Precision: all big matmuls run fp8e4 (DoubleRow, K=256/instr). The attention
correction is ~2e-4 of ||x||, so a few % of fp8 noise on it is ~1e-5 overall.

Sharding: data-parallel over batch, 2 per core, no collectives.
"""

import math
import os
import sys

import numpy as np

for _p in ("/opt/trn_rl_repo", "/opt/pypackages"):
    if os.path.isdir(_p) and _p not in sys.path:
        sys.path.append(_p)

import concourse.bass as bass
import concourse.mybir as mybir
import concourse.tile as tile
from concourse.bass_utils import run_bass_kernel_spmd

B, C, H, W = 16, 512, 32, 32
NPOS = H * W            # 1024
NCORES = 8
BPC = B // NCORES       # batches per core
KT = 4                  # 512 channels = 4 k-tiles of 128
EPS = 1e-5
SC = float(C) ** -0.5
WSCALE = 64.0           # host pre-scale on G / WVN for fp8 range
EXP_SCALE = SC / WSCALE
LN1024 = math.log(1024.0)
OUT_SCALE = 1.0 / (WSCALE * 1024.0)
XSCALE = 65536.0        # host pre-scale on x (= 1/OUT_SCALE, exact pow2)
EPS_DEV = EPS * XSCALE * XSCALE
F32 = mybir.dt.float32
F32R = mybir.dt.float32r
FP8 = mybir.dt.float8e4
AF = mybir.ActivationFunctionType
ALU = mybir.AluOpType
AX = mybir.AxisListType
DR = mybir.MatmulPerfMode.DoubleRow

# aux constant-tensor (fp32) column layout
A_F16 = 0             # [128, 8]    F16[p, g] = (p // 16 == g) / 16
A_E16 = 8             # [8, 128]    E16[g, q] = (q // 16 == g)
A_I128 = 136          # [128, 128]  identity (residual matmul)
A_ONES32 = 264        # [32, 128]   ones (diag broadcast matmul, K=32)
NAUXF = 392
# fp8 merged-const column layout (bytes)
Q_G = 0               # [128, 4*512] g rearranged (k p) n -> p (k n)
Q_WVN = 2048
Q_FIND = 4096         # [128, 2*32] f_ind pair
Q_R32H = 4160         # [32, 1024]  R32H[k, n] = (n // 32 == k)  (0/1, exact fp8)
NQ = 5184


def _r(ap):
    return ap.bitcast(F32R)


def _split_sync_waits(nc, maxw=1):
    """walrus embeds at most one sync-wait per instruction; move extra waits
    onto preceding same-queue NoOps (FIFO queues keep semantics)."""
    n = 0
    for fn in nc.m.functions:
        for blk in fn.blocks:
            out = []
            for inst in blk.instructions:
                si = inst.sync_info
                waits = list(si.on_wait) if (si is not None and si.on_wait) else []
                if len(waits) > maxw:
                    keep = waits[-maxw:]
                    extra = waits[:-maxw]
                    for i in range(0, len(extra), maxw):
                        nop = mybir.InstNoOp(name=f"wsplit-{n}")
                        n += 1
                        nop.engine = inst.engine
                        nop.sync_info = mybir.SyncInfo(
                            on_wait=extra[i:i + maxw], on_update=[]
                        )
                        out.append(nop)
                    si.on_wait = keep
                out.append(inst)
            blk.instructions = out
    return n


def _build_nc():
    nc = bass.Bass()
    x_ext = nc.declare_dram_parameter("x", [BPC, C, NPOS], F32, isOutput=False)
    aux_ext = nc.declare_dram_parameter("aux", [128, NAUXF], F32, isOutput=False)
    fq_ext = nc.declare_dram_parameter("fq", [128, NQ], FP8, isOutput=False)
    out_ext = nc.declare_dram_parameter("out", [BPC, C, NPOS], F32, isOutput=True)

    with tile.TileContext(nc) as tc:
        from contextlib import ExitStack

        with ExitStack() as ctx:
            wpool = ctx.enter_context(tc.tile_pool(name="wpool", bufs=1))
            xpool = ctx.enter_context(tc.tile_pool(name="xpool", bufs=2))
            hnpool = ctx.enter_context(tc.tile_pool(name="hnpool", bufs=2))
            hhpool = ctx.enter_context(tc.tile_pool(name="hhpool", bufs=2))
            hspool = ctx.enter_context(tc.tile_pool(name="hspool", bufs=2))
            opool = ctx.enter_context(tc.tile_pool(name="opool", bufs=2))
            epool = ctx.enter_context(tc.tile_pool(name="epool", bufs=4))
            spool = ctx.enter_context(tc.tile_pool(name="spool", bufs=2))
            ps_big = ctx.enter_context(tc.tile_pool(name="ps_big", bufs=2, space="PSUM"))
            ps_r = ctx.enter_context(tc.tile_pool(name="ps_r", bufs=1, space="PSUM"))
            ps_sm = ctx.enter_context(tc.tile_pool(name="ps_sm", bufs=2, space="PSUM"))

            fq_sb = wpool.tile([128, NQ], FP8, tag="fq_sb", name="fq_sb")
            aux_sb = wpool.tile([128, NAUXF], F32R, tag="aux_sb", name="aux_sb")
            warm_sb = wpool.tile([128, 512], F32, tag="warm_sb", name="warm_sb")
            eps_sb = wpool.tile([128, 1], F32, tag="eps_sb", name="eps_sb")
            ln1024_sb = wpool.tile([128, 1], F32, tag="ln1024_sb", name="ln1024_sb")

            g_sb = fq_sb[:, Q_G:Q_G + 2048].rearrange("p (k n) -> p k n", k=KT)
            wvn_sb = fq_sb[:, Q_WVN:Q_WVN + 2048].rearrange("p (k n) -> p k n", k=KT)
            auxq_sb = fq_sb[:, Q_FIND:Q_FIND + 64].rearrange("p (a b) -> p a b", a=2)
            f16 = aux_sb[:, A_F16:A_F16 + 8]
            e16 = aux_sb[0:8, A_E16:A_E16 + 128]
            i128 = aux_sb[:, A_I128:A_I128 + 128]
            ones32 = aux_sb[0:32, A_ONES32:A_ONES32 + 128]
            r32h = fq_sb[0:32, Q_R32H:Q_R32H + NPOS]

            st = [dict() for _ in range(BPC)]

            def warmup(n):
                nc.vector.memset(warm_sb, 0.0)
                nc.vector.memset(eps_sb, EPS_DEV)
                nc.vector.memset(ln1024_sb, LN1024)
                # dense N=512 stream: ~80% PE duty, flips the HAM clock gate
                # to 2.4GHz ~4us in (N=128 with per-MM LDWEIGHTS never does)
                wps = ps_sm.tile([128, 512], F32, tag="sm", name="sm")
                for _ in range(n):
                    nc.tensor.matmul(wps, _r(warm_sb[:, 0:128]), _r(warm_sb),
                                     start=True, stop=True)

            def filler_f32(n, rhs_ap):
                """Keep-warm matmuls gated on real data (bridges DMA waits so
                the HAM clock gate never re-throttles the PE)."""
                wf = ps_big.tile([128, 512], F32, tag="big", name="big")
                for _ in range(n):
                    nc.tensor.matmul(wf, _r(warm_sb[:, 0:128]), rhs_ap,
                                     start=True, stop=True)

            def filler_ep(n, ep):
                wf = ps_big.tile([128, 512], F32, tag="big", name="big")
                for _ in range(n):
                    nc.tensor.matmul(wf[0:32, 0:256], auxq_sb, ep[:, :, 0:256],
                                     start=True, stop=True, perf_mode=DR)

            def load_input_dmas():
                """Few big DMAs: each HWDGE transfer costs ~2.4us of serial
                ring time, so transfer count dominates the ramp."""
                for b in range(BPC):
                    st[b]["x"] = [
                        xpool.tile([128, 2, NPOS], F32R, tag=f"x_sb{h}",
                                   name=f"x_sb{h}")
                        for h in range(2)
                    ]
                # ring ACT: fp8 consts + aux (small, done early)
                nc.scalar.dma_start(out=fq_sb, in_=fq_ext[:, :])
                nc.scalar.dma_start(out=aux_sb, in_=aux_ext[:, :].bitcast(F32R))
                # ring SP: all of x, batch 0 first (it gates the whole ramp)
                xv0 = x_ext[0].bitcast(F32R).rearrange("(h p) n -> p h n", p=128)
                nc.sync.dma_start(out=st[0]["x"][0], in_=xv0[:, 0:2])
                nc.sync.dma_start(out=st[0]["x"][1], in_=xv0[:, 2:4])
                xv1 = x_ext[1].bitcast(F32R).rearrange("(h p) n -> p h n", p=128)
                nc.sync.dma_start(out=st[1]["x"][0], in_=xv1[:, 0:2])
                nc.sync.dma_start(out=st[1]["x"][1], in_=xv1[:, 2:4])

            def xkt(b, kt):
                return st[b]["x"][kt // 2][:, kt % 2]

            def stats_pair(b, h):
                """GroupNorm stats + fp8 hn cast for one x half (kts 2h,2h+1).
                Gates only on that half's DMA, so kt01 normalizes while kt23
                is still in flight."""
                s = st[b]
                if h == 0:
                    s["hn"] = hnpool.tile([128, KT, NPOS], FP8, tag="hn_sb",
                                          name="hn_sb")
                hn_sb = s["hn"]
                stats = spool.tile([128, 2, 2, 6], F32, tag=f"stats{h}",
                                   name=f"stats{h}")
                for j in range(2):
                    for sub in range(2):
                        nc.vector.bn_stats(
                            out=stats[:, j, sub, :],
                            in_=xkt(b, 2 * h + j)[:, sub * 512:(sub + 1) * 512]
                            .bitcast(F32),
                        )
                mv = spool.tile([128, 2, 2], F32, tag=f"mv{h}", name=f"mv{h}")
                for j in range(2):
                    nc.vector.bn_aggr(out=mv[:, j, :], in_=stats[:, j, :, :])
                rhs4 = spool.tile([128, 4], F32R, tag=f"rhs4_{h}",
                                  name=f"rhs4_{h}")
                nc.vector.tensor_copy(out=rhs4[:, 0:2], in_=mv[:, :, 0])
                nc.vector.tensor_tensor(
                    out=rhs4[:, 2:4], in0=mv[:, :, 0], in1=mv[:, :, 0], op=ALU.mult
                )
                nc.vector.tensor_tensor(
                    out=rhs4[:, 2:4], in0=rhs4[:, 2:4].bitcast(F32), in1=mv[:, :, 1],
                    op=ALU.add,
                )
                gst_ps = ps_sm.tile([8, 4], F32, tag="sm", name="sm")
                nc.tensor.matmul(gst_ps, _r(f16), _r(rhs4), start=True, stop=True)
                # mu_inv: cols 0:2 = -mu_g, cols 2:4 = invsigma_g (per kt)
                mu_inv = spool.tile([8, 4], F32R, tag=f"mu_inv{h}",
                                    name=f"mu_inv{h}")
                nc.scalar.mul(out=mu_inv[:, 0:2], in_=gst_ps[:, 0:2], mul=-1.0)
                m2 = spool.tile([8, 2], F32, tag=f"m2_{h}", name=f"m2_{h}")
                nc.scalar.square(out=m2, in_=gst_ps[:, 0:2])
                var2 = spool.tile([8, 2], F32, tag=f"var2_{h}", name=f"var2_{h}")
                nc.vector.tensor_tensor(
                    out=var2, in0=gst_ps[:, 2:4], in1=m2, op=ALU.subtract
                )
                lnv = spool.tile([8, 2], F32, tag=f"lnv{h}", name=f"lnv{h}")
                nc.scalar.activation(out=lnv, in_=var2, func=AF.Ln,
                                     bias=eps_sb[0:8, :])
                nc.scalar.activation(out=mu_inv[:, 2:4], in_=lnv, func=AF.Exp,
                                     scale=-0.5)
                perch_ps = ps_sm.tile([128, 4], F32, tag="sm", name="sm")
                nc.tensor.matmul(perch_ps, _r(e16), _r(mu_inv), start=True,
                                 stop=True)
                perch = spool.tile([128, 4], F32, tag=f"perch{h}",
                                   name=f"perch{h}")
                nc.vector.tensor_copy(out=perch, in_=perch_ps)
                negms = spool.tile([128, 2], F32, tag=f"negms{h}",
                                   name=f"negms{h}")
                nc.vector.tensor_tensor(
                    out=negms, in0=perch[:, 0:2], in1=perch[:, 2:4], op=ALU.mult
                )
                for j in range(2):
                    kt = 2 * h + j
                    if j == 1 or b == 1:
                        nc.vector.tensor_scalar(
                            out=hn_sb[:, kt],
                            in0=xkt(b, kt).bitcast(F32),
                            scalar1=perch[:, 2 + j:3 + j],
                            scalar2=negms[:, j:j + 1],
                            op0=ALU.mult,
                            op1=ALU.add,
                        )
                    else:
                        nc.scalar.activation(
                            out=hn_sb[:, kt],
                            in_=xkt(b, kt).bitcast(F32),
                            func=AF.Identity,
                            scale=perch[:, 2 + j:3 + j],
                            bias=negms[:, j:j + 1],
                        )

            def hhat_mt(b, mt):
                """hh[mt] = (64*G[:,mt])^T hn, fp8 DoubleRow, cast to fp8."""
                s = st[b]
                hn_sb = s["hn"]
                if mt == 0:
                    s["hh"] = hhpool.tile([128, KT, NPOS], FP8, tag="hh_sb",
                                          name="hh_sb")
                hh_sb = s["hh"]
                ps = ps_big.tile([128, NPOS], F32, tag="big", name="big")
                # g-major: the g=0 matmuls only need hn kt0/1, so they can
                # start while the last x chunks are still landing
                for g in range(2):
                    for nh in range(2):
                        sl = slice(nh * 512, (nh + 1) * 512)
                        nc.tensor.matmul(
                            ps[:, sl],
                            g_sb[:, 2 * g:2 * g + 2, mt * 128:(mt + 1) * 128],
                            hn_sb[:, 2 * g:2 * g + 2, sl],
                            start=(g == 0),
                            stop=(g == 1),
                            perf_mode=DR,
                        )
                if mt % 2 == 0:
                    nc.scalar.copy(out=hh_sb[:, mt, :], in_=ps)
                else:
                    nc.vector.tensor_copy(out=hh_sb[:, mt, :], in_=ps)

            def hhat(b):
                for mt in range(KT):
                    hhat_mt(b, mt)

            def numer(b):
                """numT = 1024*exp(sc*S[33i,33j]) via strided fp8 matmul."""
                s = st[b]
                hn_sb, hh_sb = s["hn"], s["hh"]
                nps = ps_sm.tile([32, 32], F32, tag="sm", name="sm")
                for kt in range(KT):
                    nc.tensor.matmul(
                        nps,
                        hh_sb[:, kt, 0:NPOS:33],
                        hn_sb[:, kt, 0:NPOS:33],
                        start=(kt == 0),
                        stop=(kt == KT - 1),
                    )
                s["numT"] = numT = spool.tile([32, 32], F32, tag="numT", name="numT")
                nc.scalar.activation(out=numT, in_=nps, func=AF.Exp,
                                     scale=EXP_SCALE, bias=ln1024_sb[0:32, :])

            def s_phase(b, post_nt):
                """S tiles -> exp(fp8) -> psR row-reduction (2-tile lag so the
                psR matmul never stalls the PE on the exp)."""
                s = st[b]
                hn_sb, hh_sb = s["hn"], s["hh"]
                s["psR"] = psR = ps_r.tile([32, NPOS], F32, tag="psR", name="psR")
                pairs = []
                e_pair = None

                def psr_mm(pi):
                    ep = pairs[pi]
                    for mh in range(2):
                        sl = slice(mh * 512, (mh + 1) * 512)
                        nc.tensor.matmul(
                            psR[:, sl],
                            auxq_sb,
                            ep[:, :, sl],
                            start=(pi == 0),
                            stop=(pi == 3),
                            perf_mode=DR,
                            skip_group_check=True,
                        )

                for nt in range(8):
                    ps = ps_big.tile([128, NPOS], F32, tag="big", name="big")
                    # g-major: one LDWEIGHTS serves both mh halves (PE duty up)
                    for g in range(2):
                        for mh in range(2):
                            sl = slice(mh * 512, (mh + 1) * 512)
                            nc.tensor.matmul(
                                ps[:, sl],
                                hh_sb[:, 2 * g:2 * g + 2, nt * 128:(nt + 1) * 128],
                                hn_sb[:, 2 * g:2 * g + 2, sl],
                                start=(g == 0),
                                stop=(g == 1),
                                perf_mode=DR,
                            )
                    if nt % 2 == 0:
                        e_pair = epool.tile([128, 2, NPOS], FP8, tag="e_pair",
                                            name="e_pair")
                        pairs.append(e_pair)
                    nc.scalar.activation(out=e_pair[:, nt % 2, :], in_=ps,
                                         func=AF.Exp, scale=EXP_SCALE)
                    for fn in post_nt.get(nt, []):
                        fn()
                for pi in range(4):
                    psr_mm(pi)
                s["last_ep"] = pairs[3]

            def diag_chain(b):
                """psR(PSUM) -> denT -> diagT (exact orientation, no transpose)."""
                s = st[b]
                psR, numT = s["psR"], s["numT"]
                denT = spool.tile([32, 32], F32, tag="denT", name="denT")
                nc.vector.tensor_reduce(
                    out=denT,
                    in_=psR.rearrange("p (a b) -> p b a", a=32),
                    axis=AX.X,
                    op=ALU.add,
                )
                rdenT = spool.tile([32, 32], F32, tag="rdenT", name="rdenT")
                nc.vector.reciprocal(out=rdenT, in_=denT)
                diagT = spool.tile([32, 32], F32, tag="diagT", name="diagT")
                nc.vector.tensor_tensor(out=diagT, in0=numT, in1=rdenT, op=ALU.mult)
                s["diagT"] = diagT

            def d_bcast_half(b, nh):
                """D[c, n] = diagT[n//32, n%32] broadcast: mask-multiply on the
                DVE (stride-0 broadcast read) + K=32 ones matmul. Avoids the
                SBUF->SBUF flatten DMA (~2.4us of ring serial time)."""
                s = st[b]
                diagT = s["diagT"]
                sl = slice(nh * 512, (nh + 1) * 512)
                masked = spool.tile([32, 512], F32R, tag=f"msk{nh}",
                                    name=f"msk{nh}")
                nc.vector.tensor_tensor(
                    out=masked.rearrange("p (a b) -> p a b", a=16),
                    in0=r32h[:, sl].rearrange("p (a b) -> p a b", a=16),
                    in1=diagT.unsqueeze(1).broadcast_to([32, 16, 32]),
                    op=ALU.mult,
                )
                ps_d = ps_sm.tile([128, 512], F32, tag="sm", name="sm")
                nc.tensor.matmul(ps_d, _r(ones32), masked, start=True, stop=True)
                s.setdefault("ps_d", [None, None])[nh] = ps_d

            def hs_half(b, nh):
                """hs[:, :, half] = hn * D (fp8), 4 DVE ops."""
                s = st[b]
                hn_sb = s["hn"]
                ps_d = s["ps_d"][nh]
                if nh == 0:
                    s["hs"] = hspool.tile([128, KT, NPOS], FP8, tag="hs_sb",
                                          name="hs_sb")
                hs_sb = s["hs"]
                sl = slice(nh * 512, (nh + 1) * 512)
                for kt in range(KT):
                    nc.vector.tensor_tensor(
                        out=hs_sb[:, kt, sl], in0=hn_sb[:, kt, sl], in1=ps_d,
                        op=ALU.mult,
                    )

            def out_mt(b, mt, residual_pe):
                """psum[mt] = (64*WVN)^T hs (+ 65536*x via identity matmul on
                the tail path); lands in the o2 pair tile via ACT/DVE."""
                s = st[b]
                hs_sb = s["hs"]
                ps = ps_big.tile([128, NPOS], F32, tag="big", name="big")
                for g in range(2):
                    for nh in range(2):
                        sl = slice(nh * 512, (nh + 1) * 512)
                        nc.tensor.matmul(
                            ps[:, sl],
                            wvn_sb[:, 2 * g:2 * g + 2, mt * 128:(mt + 1) * 128],
                            hs_sb[:, 2 * g:2 * g + 2, sl],
                            start=(g == 0),
                            stop=(not residual_pe and g == 1),
                            perf_mode=DR,
                        )
                if residual_pe:
                    for nh in range(2):
                        sl = slice(nh * 512, (nh + 1) * 512)
                        nc.tensor.matmul(
                            ps[:, sl], _r(i128), xkt(b, mt)[:, sl],
                            start=False, stop=True,
                        )
                if mt % 2 == 0:
                    s["o2"] = opool.tile([128, 2, NPOS], F32, tag="o_sb",
                                         name="o_sb")
                o2 = s["o2"]
                if residual_pe:
                    # halves on both engines in parallel
                    nc.scalar.copy(out=o2[:, mt % 2, 0:512], in_=ps[:, 0:512])
                    nc.vector.tensor_copy(out=o2[:, mt % 2, 512:1024],
                                          in_=ps[:, 512:1024])
                else:
                    nc.vector.tensor_tensor(out=o2[:, mt % 2, :], in0=ps,
                                            in1=xkt(b, mt).bitcast(F32),
                                            op=ALU.add)

            def out_dma(b, pair, ring):
                """One 1MB DMA per mt pair (each transfer costs ~2.4us ring)."""
                s = st[b]
                ov = out_ext[b].rearrange("(k p) n -> p k n", p=128)
                ring.dma_start(out=ov[:, 2 * pair:2 * pair + 2, :], in_=s["o2"])

            def out_dma_single(b, mt, ring):
                """0.5MB DMA for one mt (smaller final transfer on the tail)."""
                s = st[b]
                ov = out_ext[b].rearrange("(k p) n -> p k n", p=128)
                ring.dma_start(out=ov[:, mt:mt + 1, :],
                               in_=s["o2"][:, mt % 2:mt % 2 + 1, :])

            # ---- pipelined emission over the two batches ----
            warmup(int(os.environ.get("TRN_WARM_N", "24")))
            load_input_dmas()
            stats_pair(0, 0)
            filler_f32(6, xkt(0, 0)[:, 0:512])    # keep HAM warm to x0b
            stats_pair(0, 1)
            filler_f32(6, xkt(0, 2)[:, 0:512])    # keep HAM warm to hn-casts
            hhat(0)
            # batch-1 stats interleave into batch-0's S phase
            numer(0)
            # batch-1 stats + hhat + numer all absorb into batch-0's S phase
            s_phase(0, post_nt={
                2: [lambda: stats_pair(1, 0)],
                3: [lambda: stats_pair(1, 1)],
                6: [lambda: hhat_mt(1, 0), lambda: hhat_mt(1, 1)],
                7: [lambda: hhat_mt(1, 2), lambda: hhat_mt(1, 3),
                    lambda: numer(1)],
            })
            diag_chain(0)
            # batch-1 S phase interleaved with batch-0 hs/out phase
            s_phase(1, post_nt={
                0: [lambda: d_bcast_half(0, 0), lambda: hs_half(0, 0),
                    lambda: d_bcast_half(0, 1), lambda: hs_half(0, 1)],
                3: [lambda: out_mt(0, 0, False)],
                5: [lambda: out_mt(0, 1, False),
                    lambda: out_dma(0, 0, nc.sync)],
                6: [lambda: out_mt(0, 2, False)],
                7: [lambda: out_mt(0, 3, True),
                    lambda: out_dma(0, 1, nc.sync)],
            })
            filler_ep(8, st[1]["last_ep"])        # bridge diag1 chain
            diag_chain(1)
            # tail: PE-residual + ACT copy keeps the DVE off the critical path
            d_bcast_half(1, 0)
            hs_half(1, 0)
            filler_ep(8, st[1]["last_ep"])        # bridge hs latency, stay warm
            d_bcast_half(1, 1)
            hs_half(1, 1)
            out_mt(1, 0, True)
            out_mt(1, 1, True)
            out_dma(1, 0, nc.sync)
            out_mt(1, 2, True)
            out_dma_single(1, 2, nc.sync)
            out_mt(1, 3, True)
            out_dma_single(1, 3, nc.scalar)
    if os.environ.get("TRN_NO_WAITSPLIT") != "1":
        _split_sync_waits(nc, maxw=1)
    return nc


def _make_aux():
    aux = np.zeros((128, NAUXF), np.float32)
    p = np.arange(128)
    aux[p, A_F16 + (p // 16) % 8] = 1.0 / 16.0
    for g in range(8):
        for q in range(128):
            if q // 16 == g:
                aux[g, A_E16 + q] = 1.0
    aux[p, A_I128 + p] = 1.0
    aux[0:32, A_ONES32:A_ONES32 + 128] = 1.0
    return aux


def _make_fq(G, WVN, FP8NP):
    """Merged fp8 consts: g / wvn rearranged (k p) n -> p (k n), f_ind pair."""
    fq = np.zeros((128, NQ), FP8NP)
    gr = G.reshape(KT, 128, C).transpose(1, 0, 2).reshape(128, KT * C)
    wr = WVN.reshape(KT, 128, C).transpose(1, 0, 2).reshape(128, KT * C)
    fq[:, Q_G:Q_G + 2048] = gr
    fq[:, Q_WVN:Q_WVN + 2048] = wr
    p = np.arange(128)
    fq[p, Q_FIND + p % 32] = 1.0
    fq[p, Q_FIND + 32 + p % 32] = 1.0
    n = np.arange(NPOS)
    for k in range(32):
        fq[k, Q_R32H:Q_R32H + NPOS] = (n // 32 == k).astype(np.float32)
    return fq


def _reference_numpy(x, Wq, bq, Wk, bk, Wv, bv, Wn, bn):
    """Exact (slow) numpy fallback, only used if biases are nonzero."""
    Bn_, C_, H_, W_ = x.shape
    xg = x.reshape(Bn_, 32, -1).astype(np.float64)
    mu = xg.mean(-1, keepdims=True)
    var = xg.var(-1, keepdims=True)
    h = ((xg - mu) / np.sqrt(var + EPS)).reshape(Bn_, C_, H_, W_).astype(np.float32)
    bqv = bq.reshape(1, C_, 1, 1)
    bkv = bk.reshape(1, C_, 1, 1)
    bvv = bv.reshape(1, C_, 1, 1)
    bnv = bn.reshape(1, C_, 1, 1)

    def nin(t, Wm, bb):
        return np.einsum("bchw,co->bowh", t, Wm, optimize=True) + bb

    q = nin(h, Wq, bqv)
    k = nin(h, Wk, bkv)
    v = nin(h, Wv, bvv)
    out = np.empty_like(x)
    sc = C_ ** -0.5
    for bi in range(Bn_):
        Q = q[bi].transpose(2, 1, 0).reshape(-1, C_)
        K = k[bi].transpose(2, 1, 0).reshape(-1, C_)
        S = (Q @ K.T) * sc
        S5 = S.reshape(H_, W_, H_, W_).transpose(1, 3, 0, 2)
        Sm = S5.reshape(W_, W_, -1)
        Sm = Sm - Sm.max(-1, keepdims=True)
        E = np.exp(Sm)
        SMX = (E / E.sum(-1, keepdims=True)).reshape(W_, W_, H_, H_)
        ii = np.arange(H_)
        jj = np.arange(W_)
        diag = SMX[ii[:, None], jj[None, :], ii[:, None], jj[None, :]]
        h2v = v[bi] * np.swapaxes(diag, 0, 1)[None]
        out[bi] = np.einsum("cwh,co->ohw", h2v, Wn, optimize=True) + bnv[0]
    return (x + out).astype(np.float32)


_NC_CACHE = None


def kernel(**inputs):
    x = np.ascontiguousarray(np.asarray(inputs["x"], dtype=np.float32))
    Wq = np.asarray(inputs["Wq"], dtype=np.float32)
    Wk = np.asarray(inputs["Wk"], dtype=np.float32)
    Wv = np.asarray(inputs["Wv"], dtype=np.float32)
    Wn = np.asarray(inputs["Wn"], dtype=np.float32)
    bq = np.asarray(inputs["bq"], dtype=np.float32)
    bk = np.asarray(inputs["bk"], dtype=np.float32)
    bv = np.asarray(inputs["bv"], dtype=np.float32)
    bn = np.asarray(inputs["bn"], dtype=np.float32)

    if any(np.any(bb != 0) for bb in (bq, bk, bv, bn)):
        return _reference_numpy(x, Wq, bq, Wk, bk, Wv, bv, Wn, bn)

    import ml_dtypes

    FP8NP = ml_dtypes.float8_e4m3
    G = np.clip(Wq @ Wk.T * WSCALE, -240, 240).astype(FP8NP)
    WVN = np.clip(Wv @ Wn * WSCALE, -240, 240).astype(FP8NP)
    aux = _make_aux()
    fq = _make_fq(G, WVN, FP8NP)

    global _NC_CACHE
    if _NC_CACHE is None:
        _NC_CACHE = _build_nc()
    nc = _NC_CACHE

    xf = (x * XSCALE).reshape(B, C, NPOS)   # exact pow2 scale, undone on device
    in_maps = [
        {
            "x": np.ascontiguousarray(xf[c * BPC:(c + 1) * BPC]),
            "aux": aux,
            "fq": fq,
        }
        for c in range(NCORES)
    ]
    trace = bool(int(os.environ.get("TRN_KERNEL_TRACE", "0")))
    res = run_bass_kernel_spmd(nc, in_maps, core_ids=list(range(NCORES)), trace=trace)
    if trace:
        kernel.last_exec_time_ns = res.exec_time_ns
        kernel.last_results = res
    out = np.empty((B, C, NPOS), np.float32)
    for c in range(NCORES):
        # device emits 65536*(x + correction); undo the exact pow2 scale
        out[c * BPC:(c + 1) * BPC] = res.results[c]["out"]
    out *= OUT_SCALE
    return out.reshape(B, C, H, W)
